# revision 8
# baseline (speedup 1.0000x reference)
"""GATv2 message-passing model (2 layers, fwd+bwd GAT + merge MLP + BN) on 8 TRN2 cores.

Strategy (edge-parallel, dst-sorted):
  - Edges of each direction are sorted by destination node and sharded across
    the 8 cores by contiguous dst ranges (1250 nodes/core, padded to 1344).
  - Per core, dst nodes are grouped into blocks of 112; each block's edges are
    packed into 128-edge tiles.  A combined matmul with lhsT=[edge_attr^T ;
    one_hot(dst)^T] (K=16+112=128) produces ee+xr per edge tile straight into
    PSUM; the gathered source features xl[src] (indirect DMA from a
    bf16 DRAM table) are injected with an identity matmul.
  - The HC axis is head-interleaved ((c,h), h fastest) so that per-edge
    per-head scalars are packed along the innermost free dim -> DVE 2x/4x.
  - Per-block batched DVE: att-mul + fold-tree + grouped reduce for logits
    (fp16), one batched Exp, one batched alpha-weighting of gx.
  - Unnormalised aggregation: scatter-matmuls with lhsT = one_hot(dst)
    accumulate exp-weighted gx into a per-block PSUM; normalisation by the
    segment sum happens per node block (divide by denom, mean over heads).
  - Node-level work (xl/xr/merge-MLP/BN) is node-sharded; xl is AllGathered
    (bf16) per layer/direction; BN stats use a tiny AllReduce.
"""

import os
import sys
from contextlib import ExitStack

import numpy as np
import ml_dtypes

for _p in ("/opt/trn_rl_repo",):
    if _p not in sys.path and os.path.isdir(_p):
        sys.path.append(_p)

import concourse.bass as bass
import concourse.bacc as bacc
import concourse.tile as tile
from concourse import mybir
from concourse.masks import make_identity
from concourse import bass_utils

BF16 = ml_dtypes.bfloat16
F16 = np.float16
F32 = np.float32
DT = mybir.dt
ALU = mybir.AluOpType

NEG_SLOPE = 0.2
EPS = 1e-5


# ----------------------------------------------------------------------------
# Config
# ----------------------------------------------------------------------------
class Cfg:
    def __init__(self, N=10000, E=100000, DIN=128, H=4, C=128, ED=16, L=2, NC=8):
        assert DIN == 128 and H * C == 512 and ED == 16
        self.N, self.E, self.DIN, self.H, self.C, self.ED, self.L, self.NC = (
            N, E, DIN, H, C, ED, L, NC)
        self.HC = H * C
        self.NPC = (N + NC - 1) // NC          # real nodes per core
        self.BLK = 112                          # dst-nodes per scatter block
        self.NB = (self.NPC + self.BLK - 1) // self.BLK   # blocks per core
        self.NPAD = self.NB * self.BLK          # padded nodes per core
        self.NTAB = NC * self.NPAD              # rows in gathered xl tables


def _perm_hc(cfg):
    """hc column permutation: interleaved j=c*H+h <- original h*C+c."""
    h = np.arange(cfg.H)
    c = np.arange(cfg.C)
    # perm[j] = original index stored at interleaved position j
    return (c[:, None] + h[None, :] * cfg.C).reshape(-1)  # [C*H] -> orig idx


# ----------------------------------------------------------------------------
# Host-side preprocessing
# ----------------------------------------------------------------------------
def _prep_direction(cfg, edges, edge_attr):
    """Sort/shard/block/tile the edges of one direction.

    Returns (tiles_per_block [NB], per-core dict arrays).
    """
    NC, NPC, BLK, NB = cfg.NC, cfg.NPC, cfg.BLK, cfg.NB
    src, dst = np.asarray(edges[0]), np.asarray(edges[1])
    order = np.argsort(dst, kind="stable")
    s_src, s_dst = src[order], dst[order]
    core = np.minimum(s_dst // NPC, NC - 1)

    # edge counts per (core, block)
    counts = np.zeros((NC, NB), dtype=np.int64)
    per_core_edges = []
    for k in range(NC):
        sel = core == k
        ls, ld = s_src[sel], s_dst[sel] - k * NPC
        ea = edge_attr[order[sel]]
        blk = ld // BLK
        per_core_edges.append((ls, ld, ea, blk))
        cb = np.bincount(blk, minlength=NB)
        counts[k, : len(cb)] = cb[:NB]

    tiles_per_block = np.maximum(1, -(-counts.max(axis=0) // 128))  # ceil
    TT = int(tiles_per_block.sum())
    t_off = np.concatenate([[0], np.cumsum(tiles_per_block)]).astype(np.int64)

    out = []
    for k in range(NC):
        ls, ld, ea, blk = per_core_edges[k]
        gidx = np.zeros((128, TT), dtype=np.int32)
        comb = np.zeros((TT, 128, 128), dtype=BF16)
        scat = np.zeros((TT, 128, BLK), dtype=BF16)
        for b in range(NB):
            sel = blk == b
            nsel = int(sel.sum())
            if nsel == 0:
                continue
            j = np.arange(nsel)
            t = t_off[b] + j // 128
            p = j % 128
            srcs = ls[sel]
            # row index in the padded xl table
            rows = (srcs // NPC) * cfg.NPAD + (srcs % NPC)
            gidx[p, t] = rows.astype(np.int32)
            comb[t, 112:128, p] = ea[sel].astype(BF16)
            loc = (ld[sel] - b * BLK).astype(np.int64)
            comb[t, loc, p] = BF16(1.0)
            scat[t, p, loc] = BF16(1.0)
        # int16 wrapped+replicated index layout for dma_gather: index
        # i = t*128 + p (tile-major) lives at [i % 16, i // 16], with the
        # 16-row block replicated across all 8 gpsimd core slices.
        flat = gidx.T.reshape(-1)  # i = t*128 + p
        ncol = TT * 128 // 16
        blk16 = flat.reshape(ncol, 16).T.astype(np.int16)
        gidx16 = np.tile(blk16, (8, 1))
        out.append(dict(gidx16=gidx16, comb=comb, scat=scat))
    return [int(x) for x in tiles_per_block], out


def preprocess(cfg, inputs):
    """Build per-core in_maps + meta (tile counts)."""
    NC, NPC, NPAD, L = cfg.NC, cfg.NPC, cfg.NPAD, cfg.L
    x = np.asarray(inputs["x"], dtype=F32)
    perm = _perm_hc(cfg)

    meta = {}
    per_core = [dict() for _ in range(NC)]

    # node features, transposed + padded, per core
    for k in range(NC):
        xs = x[k * NPC: min((k + 1) * NPC, cfg.N)]
        xt = np.zeros((cfg.DIN, NPAD), dtype=BF16)
        xt[:, : xs.shape[0]] = xs.T.astype(BF16)
        per_core[k]["hT0"] = xt

    for d, ekey in (("f", "fwd_edges_index"), ("b", "bwd_edges_index")):
        tpb, arrs = _prep_direction(cfg, np.asarray(inputs[ekey]),
                                    np.asarray(inputs["edge_attr"], dtype=F32))
        meta[f"tpb_{d}"] = tpb
        for k in range(NC):
            per_core[k][f"gidx16_{d}"] = arrs[k]["gidx16"]
            per_core[k][f"comb_{d}"] = arrs[k]["comb"]
            per_core[k][f"scat_{d}"] = arrs[k]["scat"]

        # weights for this direction (replicated on all cores); hc axis
        # head-interleaved via perm.
        Wl = np.asarray(inputs[f"Wl_{d}"], dtype=F32)[:, :, perm]
        Wr = np.asarray(inputs[f"Wr_{d}"], dtype=F32)[:, :, perm]
        We = np.asarray(inputs[f"We_{d}"], dtype=F32)[:, :, perm]
        att = np.asarray(inputs[f"att_{d}"], dtype=F32)     # [L,H,C]
        gb = np.asarray(inputs[f"bias_{d}"], dtype=F32)     # [L,C]
        bl = np.asarray(inputs[f"bl_{d}"], dtype=F32)[:, perm]
        br = np.asarray(inputs[f"br_{d}"], dtype=F32)[:, perm]
        meta[f"has_sbias_{d}"] = bool(np.any(bl) or np.any(br))

        # layer-0 gather table precomputed on host (x is replicated, so no
        # AllGather is needed for the first layer): row k*NPAD+j = bf16(x_kj)
        # @ bf16(Wl[0]) in fp32, stored bf16 -- matching the PE numerics.
        xb = x.astype(BF16).astype(F32)
        wb = Wl[0].astype(BF16).astype(F32)
        xl0 = (xb @ wb).astype(BF16)                      # [N, HC] interleaved
        tab0 = np.zeros((cfg.NTAB, cfg.HC), dtype=BF16)
        for k in range(NC):
            nk = min((k + 1) * NPC, cfg.N) - k * NPC
            tab0[k * NPAD: k * NPAD + nk] = xl0[k * NPC: k * NPC + nk]
        for k in range(NC):
            per_core[k][f"xtab0_{d}"] = tab0

        attil = att.reshape(L, cfg.HC)[:, perm]             # interleaved att
        attb = np.broadcast_to(attil.reshape(L, 1, cfg.HC), (L, 128, cfg.HC))
        sb = (bl + br).reshape(L, 1, cfg.HC)
        sbias = np.broadcast_to(sb, (L, 128, cfg.HC))
        # msg path misses the +bl of xl = h@Wl + bl; after softmax-normalise
        # and head-mean that is exactly +mean_h(bl) per channel -> fold into
        # the GAT output bias.
        gb = gb + np.asarray(inputs[f"bl_{d}"], dtype=F32)\
            .reshape(L, cfg.H, cfg.C).mean(axis=1)
        for k in range(NC):
            per_core[k][f"Wl_{d}"] = Wl.astype(BF16)
            per_core[k][f"Wr_{d}"] = Wr.astype(BF16)
            per_core[k][f"We_{d}"] = We.astype(BF16)
            per_core[k][f"attb_{d}"] = np.ascontiguousarray(attb.astype(F16))
            per_core[k][f"gatb_{d}"] = np.ascontiguousarray(gb.reshape(L, cfg.C, 1))
            if meta[f"has_sbias_{d}"]:
                per_core[k][f"sbias_{d}"] = np.ascontiguousarray(sbias.astype(F32))

    Wm1 = np.asarray(inputs["Wm1"], dtype=F32)   # [L, 2C, C]
    Wm2 = np.asarray(inputs["Wm2"], dtype=F32)   # [L, C, C]
    bm1 = np.asarray(inputs["bm1"], dtype=F32)   # [L, C]
    gamma = np.asarray(inputs["gamma"], dtype=F32)
    beta = np.asarray(inputs["beta"], dtype=F32)
    # bm2 is dropped: BN is shift-invariant.
    for k in range(NC):
        per_core[k]["Wm1f"] = Wm1[:, : cfg.C].astype(BF16)
        per_core[k]["Wm1b"] = Wm1[:, cfg.C:].astype(BF16)
        per_core[k]["Wm2"] = Wm2.astype(BF16)
        per_core[k]["bm1"] = np.ascontiguousarray(bm1.reshape(L, cfg.C, 1))
        per_core[k]["gamma"] = np.ascontiguousarray(gamma.reshape(L, cfg.C, 1))
        per_core[k]["beta"] = np.ascontiguousarray(beta.reshape(L, cfg.C, 1))
    return per_core, meta


# ----------------------------------------------------------------------------
# Program builder
# ----------------------------------------------------------------------------
def build_program(cfg, meta, in_shapes):
    """Emit the full 2-layer program.  Same program for all 8 cores (SPMD)."""
    NC, NB, BLK, NPAD, NPC, HC, L, H, C = (cfg.NC, cfg.NB, cfg.BLK, cfg.NPAD,
                                           cfg.NPC, cfg.HC, cfg.L, cfg.H, cfg.C)
    nc = bacc.Bacc("TRN2", target_bir_lowering=False, debug=False,
                   num_devices=NC)
    rg = [list(range(NC))]

    DT_MAP = {np.dtype(np.float32): DT.float32, np.dtype(BF16): DT.bfloat16,
              np.dtype(F16): DT.float16,
              np.dtype(np.int32): DT.int32, np.dtype(np.int16): DT.int16}
    inp = {}
    for name, (shape, dt) in in_shapes.items():
        inp[name] = nc.dram_tensor(name, list(shape), DT_MAP[np.dtype(dt)],
                                   kind="ExternalInput").ap()
    out_dram = nc.dram_tensor("out", [NPC, cfg.DIN], DT.float32,
                              kind="ExternalOutput").ap()

    # internal DRAM (layer-0 gather tables come precomputed from the host)
    xtab = {}
    xloc = {}
    for d in "fb":
        xtab[0, d] = inp[f"xtab0_{d}"]
    for l in range(1, L):
        for d in "fb":
            xtab[l, d] = nc.dram_tensor(f"xtab{l}{d}", [cfg.NTAB, HC],
                                        DT.bfloat16, kind="Internal",
                                        addr_space="Shared").ap()
            xloc[l, d] = nc.dram_tensor(f"xloc{l}{d}", [NPAD, HC],
                                        DT.bfloat16, kind="Internal").ap()
    bn_in = [nc.dram_tensor(f"bnin{l}", [cfg.C, 2], DT.float32,
                            kind="Internal").ap() for l in range(L)]
    bn_out = [nc.dram_tensor(f"bnout{l}", [cfg.C, 2], DT.float32,
                             kind="Internal", addr_space="Shared").ap()
              for l in range(L)]

    with tile.TileContext(nc) as tc, ExitStack() as ctx:
        sb = ctx.enter_context(tc.tile_pool(name="sb", bufs=1))
        sb2 = ctx.enter_context(tc.tile_pool(name="sb2", bufs=2))
        sb3 = ctx.enter_context(tc.tile_pool(name="sb3", bufs=4))
        ps = ctx.enter_context(tc.tile_pool(name="ps", bufs=2, space="PSUM"))
        pse = ctx.enter_context(tc.tile_pool(name="pse", bufs=2, space="PSUM"))

        # constants
        alpha_sb = sb.tile([128, 1], DT.float32, name="alpha")
        nc.vector.memset(alpha_sb[:], NEG_SLOPE)
        id_bf = sb.tile([128, 128], DT.bfloat16, name="id_bf")
        id_f32 = sb.tile([128, 128], DT.float32, name="id_f32")
        make_identity(nc, id_bf[:])
        make_identity(nc, id_f32[:])

        # persistent SBUF state across a layer
        hT = sb.tile([128, NPAD], DT.bfloat16, name="hT", bufs=2)
        nc.sync.dma_start(hT[:], inp["hT0"][:, :])

        gidx_sb = {}
        for d in "fb":
            TT = sum(meta[f"tpb_{d}"])
            gidx_sb[d] = sb.tile([128, TT * 8], DT.int16, name=f"gidx{d}")
            nc.sync.dma_start(gidx_sb[d][:], inp[f"gidx16_{d}"][:, :])

        fT_all = {d: sb.tile([128, NPAD], DT.bfloat16, name=f"fT_{d}", bufs=2)
                  for d in "fb"}

        def node_phase(l, d, hT_cur):
            """xr -> comb_rhs rows 0:112 (+We rows 112:128); for l>0 also
            xl -> xloc (DRAM).  Returns (comb_rhs, issue_ag)."""
            Wr_sb = sb2.tile([128, HC], DT.bfloat16, tag="wr")
            nc.sync.dma_start(Wr_sb[:], inp[f"Wr_{d}"][l])
            if l > 0:
                Wl_sb = sb2.tile([128, HC], DT.bfloat16, tag="wl")
                nc.sync.dma_start(Wl_sb[:], inp[f"Wl_{d}"][l])
            comb_rhs = sb2.tile([128, NB * HC], DT.bfloat16, tag=f"crhs{d}")
            for b in range(NB):
                nc.sync.dma_start(comb_rhs[112:128, b * HC:(b + 1) * HC],
                                  inp[f"We_{d}"][l])
            if l > 0:
                xl_all = sb2.tile([BLK, NB * HC], DT.bfloat16, tag="xlall")
            for b in range(NB):
                sl = slice(b * BLK, (b + 1) * BLK)
                if l > 0:
                    ps_n = ps.tile([BLK, HC], DT.float32, tag="scr",
                                   space="PSUM", bufs=1)
                    nc.tensor.matmul(out=ps_n[:], lhsT=hT_cur[:, sl],
                                     rhs=Wl_sb[:], start=True, stop=True)
                    nc.scalar.activation(xl_all[:, b * HC:(b + 1) * HC],
                                         ps_n[:],
                                         mybir.ActivationFunctionType.Identity)

                ps_n2 = ps.tile([BLK, HC], DT.float32, tag="scr", space="PSUM", bufs=1)
                nc.tensor.matmul(out=ps_n2[:], lhsT=hT_cur[:, sl], rhs=Wr_sb[:],
                                 start=True, stop=True)
                nc.scalar.activation(
                    comb_rhs[0:BLK, b * HC:(b + 1) * HC], ps_n2[:],
                    mybir.ActivationFunctionType.Identity)

            def issue_ag():
                if l == 0:
                    return
                nc.sync.dma_start(
                    xloc[l, d][:, :].rearrange("(b p) c -> p b c", b=NB),
                    xl_all[:].rearrange("p (b c) -> p b c", b=NB))
                # AllGather local xl into the shared table
                nc.gpsimd.collective_compute(
                    "AllGather", mybir.AluOpType.bypass, replica_groups=rg,
                    ins=[xloc[l, d][:, :]], outs=[xtab[l, d][:, :]])
            return comb_rhs, issue_ag

        TMAX = max(max(meta["tpb_f"]), max(meta["tpb_b"]))

        def gather_blk(l, d, b, t0, Tb):
            gx = sb2.tile([128, TMAX * HC], DT.bfloat16, tag="gx", bufs=4)
            nc.gpsimd.dma_gather(
                gx[:, 0:Tb * HC].rearrange("p (t c) -> p t c", c=HC),
                xtab[l, d][:, :],
                gidx_sb[d][:, t0 * 8:(t0 + Tb) * 8],
                Tb * 128, Tb * 128, HC, single_packet=False)
            return gx

        def edge_phase(l, d, comb_rhs, pregx=None):
            """Per-block gather + attention + aggregation -> fT_all[d]."""
            tpb = meta[f"tpb_{d}"]
            attb_sb = sb2.tile([128, HC], DT.float16, tag="attb")
            nc.sync.dma_start(attb_sb[:], inp[f"attb_{d}"][l])
            gatb_sb = sb2.tile([C, 1], DT.float32, tag="gatb")
            nc.sync.dma_start(gatb_sb[:], inp[f"gatb_{d}"][l])
            sbias_sb = None
            if meta[f"has_sbias_{d}"]:
                sbias_sb = sb2.tile([128, HC], DT.float32, tag="sbias")
                nc.sync.dma_start(sbias_sb[:], inp[f"sbias_{d}"][l])
            t0 = 0
            for b in range(NB):
                Tb = tpb[b]
                if pregx is not None and b in pregx:
                    gx = pregx[b]
                else:
                    gx = gather_blk(l, d, b, t0, Tb)
                compT = sb2.tile([128, TMAX * 128], DT.bfloat16, tag="compT")
                nc.sync.dma_start(
                    compT[:, 0:Tb * 128].rearrange("p (t q) -> p t q", t=Tb),
                    inp[f"comb_{d}"][t0:t0 + Tb].rearrange("t p q -> p t q"))
                scatT = sb2.tile([128, TMAX * BLK], DT.bfloat16, tag="scatT")
                nc.sync.dma_start(
                    scatT[:, 0:Tb * BLK].rearrange("p (t q) -> p t q", t=Tb),
                    inp[f"scat_{d}"][t0:t0 + Tb].rearrange("t p q -> p t q"))

                # ---- phase A: z matmuls + Prelu (2-tile batched) -> lz fp16
                lz = sb3.tile([128, TMAX * HC], DT.float16, tag="lz", bufs=2)
                for tp in range(0, Tb, 2):
                    nt = min(2, Tb - tp)
                    ps_s = pse.tile([128, 1024], DT.float32, tag="zps")
                    for q in range(nt):
                        t = tp + q
                        zslice = ps_s[:, q * HC:(q + 1) * HC]
                        nc.tensor.matmul(
                            out=zslice, lhsT=compT[:, t * 128:(t + 1) * 128],
                            rhs=comb_rhs[:, b * HC:(b + 1) * HC],
                            start=True, stop=False)
                        nc.tensor.matmul(
                            out=zslice, lhsT=id_bf[:],
                            rhs=gx[:, t * HC:(t + 1) * HC],
                            start=False, stop=True)
                    dst = lz[:, tp * HC:(tp + nt) * HC]
                    src = ps_s[:, 0:nt * HC]
                    if sbias_sb is not None:
                        zf = sb3.tile([128, 2 * HC], DT.float32, tag="zf")
                        nc.vector.tensor_add(
                            zf[:, 0:nt * HC].rearrange("p (t c) -> p t c", c=HC),
                            src.rearrange("p (t c) -> p t c", c=HC),
                            sbias_sb[:].rearrange("p (o c) -> p o c", o=1)
                            .to_broadcast([128, nt, HC]))
                        src = zf[:, 0:nt * HC]
                    nc.scalar.activation(dst, src,
                                         mybir.ActivationFunctionType.Prelu,
                                         alpha=alpha_sb[:])

                # ---- phase B: batched logits on DVE (att-mul, folds, reduce)
                # att-mul in place: lz *= attb  (fp16, packed -> fast mode)
                lzv = lz[:, 0:Tb * HC].rearrange("p (t c) -> p t c", c=HC)
                nc.vector.scalar_tensor_tensor(
                    lzv, lzv, 1.0,
                    attb_sb[:].rearrange("p (o c) -> p o c", o=1)
                    .to_broadcast([128, Tb, HC]),
                    op0=ALU.mult, op1=ALU.mult)
                # fold-tree 512 -> 8 (within each tile, in place, h stays
                # packed innermost so every fold is a fast packed add)
                w = HC // 2
                while w > H:
                    nc.vector.scalar_tensor_tensor(
                        lzv[:, :, 0:w], lzv[:, :, 0:w], 1.0, lzv[:, :, w:2 * w],
                        op0=ALU.mult, op1=ALU.add)
                    w //= 2
                # last fold 8 -> 4 writes the f32 logits
                logit = sb3.tile([128, TMAX * H], DT.float32, tag="logit")
                nc.vector.scalar_tensor_tensor(
                    logit[:, 0:Tb * H].rearrange("p (t h) -> p t h", h=H),
                    lzv[:, :, 0:H], 1.0, lzv[:, :, H:2 * H],
                    op0=ALU.mult, op1=ALU.add)
                # batched exp -> exl bf16
                exl = sb3.tile([128, TMAX * H], DT.bfloat16, tag="exl")
                nc.scalar.activation(exl[:, 0:Tb * H], logit[:, 0:Tb * H],
                                     mybir.ActivationFunctionType.Exp)

                # ---- phase C: weight gx by exl (per tile, 3D views), then
                # agg+den matmuls
                for t in range(Tb):
                    gxv = gx[:, t * HC:(t + 1) * HC].rearrange(
                        "p (c h) -> p c h", h=H)
                    nc.vector.scalar_tensor_tensor(
                        gxv, gxv, 1.0,
                        exl[:, t * H:(t + 1) * H]
                        .rearrange("p (o h) -> p o h", o=1)
                        .to_broadcast([128, C, H]),
                        op0=ALU.mult, op1=ALU.mult)
                agg_ps = ps.tile([BLK, HC], DT.float32, tag="agg", space="PSUM")
                den_ps = ps.tile([BLK, H], DT.float32, tag="den", space="PSUM",
                                 bufs=1)
                for t in range(Tb):
                    scat_t = scatT[:, t * BLK:(t + 1) * BLK]
                    nc.tensor.matmul(
                        out=agg_ps[:], lhsT=scat_t,
                        rhs=gx[:, t * HC:(t + 1) * HC],
                        start=(t == 0), stop=(t == Tb - 1))
                    nc.tensor.matmul(
                        out=den_ps[:], lhsT=scat_t,
                        rhs=exl[:, t * H:(t + 1) * H],
                        start=(t == 0), stop=(t == Tb - 1))

                # ---- block tail: normalise + head mean (interleaved layout)
                den_sb = sb3.tile([BLK, H], DT.float32, tag="densb")
                nc.vector.tensor_scalar_add(den_sb[:], den_ps[:], 1e-30)
                dr = sb3.tile([BLK, H], DT.float32, tag="dr")
                nc.vector.reciprocal(dr[:], den_sb[:])
                aggN = sb3.tile([BLK, HC], DT.float32, tag="aggN", bufs=2)
                nc.vector.tensor_mul(
                    aggN[:].rearrange("p (c h) -> p c h", h=H),
                    agg_ps[:].rearrange("p (c h) -> p c h", h=H),
                    dr[:].rearrange("p (o h) -> p o h", o=1)
                    .to_broadcast([BLK, C, H]))
                fmean = sb3.tile([BLK, C], DT.float32, tag="fmean", bufs=2)
                nc.vector.reduce_sum(
                    fmean[:].rearrange("p (c o) -> p c o", o=1),
                    aggN[:].rearrange("p (c h) -> p c h", h=H),
                    axis=mybir.AxisListType.X)
                fT_ps = ps.tile([C, BLK], DT.float32, tag="scr", space="PSUM", bufs=1)
                nc.tensor.transpose(fT_ps[:], fmean[:], id_f32[0:BLK, 0:BLK])
                nc.scalar.activation(
                    fT_all[d][:, b * BLK:(b + 1) * BLK], fT_ps[:],
                    mybir.ActivationFunctionType.Identity,
                    bias=gatb_sb[:], scale=1.0 / H)
                t0 += Tb

        def merge_bn(l, hT_next):
            Wm1f_sb = sb2.tile([C, C], DT.bfloat16, tag="wm1f")
            Wm1b_sb = sb2.tile([C, C], DT.bfloat16, tag="wm1b")
            Wm2_sb = sb2.tile([C, C], DT.bfloat16, tag="wm2")
            bm1_sb = sb2.tile([C, 1], DT.float32, tag="bm1")
            nc.sync.dma_start(Wm1f_sb[:], inp["Wm1f"][l])
            nc.sync.dma_start(Wm1b_sb[:], inp["Wm1b"][l])
            nc.sync.dma_start(Wm2_sb[:], inp["Wm2"][l])
            nc.sync.dma_start(bm1_sb[:], inp["bm1"][l])
            y_all = sb2.tile([C, NPAD], DT.float32, tag="yall")
            for b in range(NB):
                sl = slice(b * BLK, (b + 1) * BLK)
                ps_m = ps.tile([C, BLK], DT.float32, tag="scr", space="PSUM", bufs=1)
                nc.tensor.matmul(out=ps_m[:], lhsT=Wm1f_sb[:],
                                 rhs=fT_all["f"][:, sl], start=True, stop=False)
                nc.tensor.matmul(out=ps_m[:], lhsT=Wm1b_sb[:],
                                 rhs=fT_all["b"][:, sl], start=False, stop=True)
                mT = sb3.tile([C, BLK], DT.bfloat16, tag="mT", bufs=2)
                nc.scalar.activation(mT[:], ps_m[:],
                                     mybir.ActivationFunctionType.Relu,
                                     bias=bm1_sb[:])
                ps_y = ps.tile([C, BLK], DT.float32, tag="scr", space="PSUM", bufs=1)
                nc.tensor.matmul(out=ps_y[:], lhsT=Wm2_sb[:], rhs=mT[:],
                                 start=True, stop=True)
                nc.scalar.activation(y_all[:, sl], ps_y[:],
                                     mybir.ActivationFunctionType.Identity)
            # BN stats over the real nodes
            stats = sb3.tile([C, 2], DT.float32, tag="stats")
            nc.vector.reduce_sum(stats[:, 0:1], y_all[:, 0:NPC],
                                 axis=mybir.AxisListType.X)
            sqscr = sb3.tile([C, NPC], DT.float32, tag="sqscr", bufs=1)
            nc.scalar.activation(sqscr[:], y_all[:, 0:NPC],
                                 mybir.ActivationFunctionType.Square,
                                 accum_out=stats[:, 1:2])
            nc.sync.dma_start(bn_in[l][:, :], stats[:])
            nc.gpsimd.collective_compute(
                "AllReduce", mybir.AluOpType.add, replica_groups=rg,
                ins=[bn_in[l][:, :]], outs=[bn_out[l][:, :]])
            stg = sb3.tile([C, 2], DT.float32, tag="stg")
            nc.sync.dma_start(stg[:], bn_out[l][:, :])
            gam = sb3.tile([C, 1], DT.float32, tag="gam")
            bet = sb3.tile([C, 1], DT.float32, tag="bet")
            nc.sync.dma_start(gam[:], inp["gamma"][l])
            nc.sync.dma_start(bet[:], inp["beta"][l])
            mu = sb3.tile([C, 1], DT.float32, tag="mu")
            nc.vector.tensor_scalar_mul(mu[:], stg[:, 0:1], 1.0 / cfg.N)
            ex2 = sb3.tile([C, 1], DT.float32, tag="ex2")
            nc.vector.tensor_scalar_mul(ex2[:], stg[:, 1:2], 1.0 / cfg.N)
            mu2 = sb3.tile([C, 1], DT.float32, tag="mu2")
            nc.vector.tensor_mul(mu2[:], mu[:], mu[:])
            var = sb3.tile([C, 1], DT.float32, tag="var")
            nc.vector.tensor_tensor(var[:], ex2[:], mu2[:],
                                    op=mybir.AluOpType.subtract)
            vare = sb3.tile([C, 1], DT.float32, tag="vare")
            nc.vector.tensor_scalar_add(vare[:], var[:], EPS)
            sd = sb3.tile([C, 1], DT.float32, tag="sd")
            nc.scalar.activation(sd[:], vare[:],
                                 mybir.ActivationFunctionType.Sqrt)
            rstd = sb3.tile([C, 1], DT.float32, tag="rstd")
            nc.vector.reciprocal(rstd[:], sd[:])
            scale = sb3.tile([C, 1], DT.float32, tag="scale")
            nc.vector.tensor_mul(scale[:], rstd[:], gam[:])
            nmu = sb3.tile([C, 1], DT.float32, tag="nmu")
            nc.vector.tensor_mul(nmu[:], mu[:], scale[:])
            bias = sb3.tile([C, 1], DT.float32, tag="bias")
            nc.vector.tensor_tensor(bias[:], bet[:], nmu[:],
                                    op=mybir.AluOpType.subtract)
            nc.scalar.activation(hT_next[:], y_all[:],
                                 mybir.ActivationFunctionType.Relu,
                                 bias=bias[:], scale=scale[:])

        # ------------------- main flow -------------------
        PREGATHER = 3
        hT_cur = hT
        for l in range(L):
            comb_rhs_f, ag_f = node_phase(l, "f", hT_cur)
            ag_f()
            comb_rhs_b, ag_b = node_phase(l, "b", hT_cur)
            # hoist the first f-gathers ahead of AG_b on the gpsimd queue so
            # edge_f compute overlaps the AG_b collective
            tpb_f = meta["tpb_f"]
            pregx = {}
            t0 = 0
            for b in range(PREGATHER):
                pregx[b] = gather_blk(l, "f", b, t0, tpb_f[b])
                t0 += tpb_f[b]
            ag_b()
            edge_phase(l, "f", comb_rhs_f, pregx)
            edge_phase(l, "b", comb_rhs_b)
            hdt = DT.bfloat16 if l < L - 1 else DT.float32
            hT_next = sb.tile([128, NPAD], hdt, name="hT", bufs=2)
            merge_bn(l, hT_next)
            hT_cur = hT_next

        # final transpose + output
        out_all = sb.tile([BLK, NB * 128], DT.float32, name="out_all")
        for b in range(NB):
            tp = ps.tile([BLK, 128], DT.float32, tag="scr", space="PSUM", bufs=1)
            nc.tensor.transpose(tp[:], hT_cur[:, b * BLK:(b + 1) * BLK], id_f32[:])
            nc.scalar.activation(out_all[:, b * 128:(b + 1) * 128], tp[:],
                                 mybir.ActivationFunctionType.Identity)
        nfull = NPC // BLK  # full blocks
        nc.sync.dma_start(
            out_dram[0:nfull * BLK].rearrange("(b p) c -> p b c", b=nfull),
            out_all[:, 0:nfull * 128].rearrange("p (b c) -> p b c", b=nfull))
        tail = NPC - nfull * BLK
        if tail > 0:
            nc.sync.dma_start(out_dram[nfull * BLK:NPC],
                              out_all[0:tail, nfull * 128:(nfull + 1) * 128])

    nc.compile()
    return nc


# ----------------------------------------------------------------------------
# Entry point
# ----------------------------------------------------------------------------
_CACHE = {}


def _run(cfg, inputs):
    per_core, meta = preprocess(cfg, inputs)
    in_shapes = {k: (v.shape, v.dtype) for k, v in per_core[0].items()}
    key = (cfg.N, cfg.E, tuple(meta["tpb_f"]), tuple(meta["tpb_b"]),
           meta["has_sbias_f"], meta["has_sbias_b"])
    if key not in _CACHE:
        _CACHE[key] = build_program(cfg, meta, in_shapes)
    nc = _CACHE[key]
    res = bass_utils.run_bass_kernel_spmd(nc, per_core,
                                          core_ids=list(range(cfg.NC)))
    outs = [res.results[k]["out"][: cfg.NPC] for k in range(cfg.NC)]
    full = np.concatenate(outs, axis=0)[: cfg.N].astype(np.float32)
    return full, nc, per_core, meta


def kernel(**inputs) -> np.ndarray:
    cfg = Cfg(N=int(inputs["x"].shape[0]), E=int(inputs["edge_attr"].shape[0]))
    out, _, _, _ = _run(cfg, inputs)
    return out
